# revision 19
# baseline (speedup 1.0000x reference)
"""MoE top-k routing + grouped down-proj GEMM + reduce-scatter for trn2 (8 cores).

Problem: intermediate_states [4, 2048, 1024] f16 (rank-sharded expanded-token
activations), w [4, 8, 1024, 2048] f16 (rank-sharded per-expert down-proj),
router_logits [1024, 8] f32, topk=2.  Output [4, 256, 2048] f16.

Strategy (W-stationary streaming): per expanded token tk routed to expert
e(tk): y[tk] = gate(tk) * (x_full[tk] @ W_full[e(tk)]) with x_full [TK, 4096]
(rank dim folded into contraction) and W_full[e] [4096, 2048].  Gates are
pre-applied to x on the host (f32 multiply, cast to f16), so the device does
pure GEMM.

Work unit = (expert, K-half, token-chunk<=512).  For 8 experts with c_e<=512
that is 16 units.  Units are sorted by token count and packed into U=2 "slots"
of 8 (one unit per core per slot); each slot has a common padded capacity C_s
= max count in slot, so all 8 cores run an IDENTICAL program (SPMD) perfectly
balanced by construction.  Per unit the PE runs, for each of 16 h-chunks of
128 output columns, a 16-step accumulation over K-subtiles:

    matmul(psum[128h, C], lhsT=W[128k, 128h], rhs=x[128k, C])

i.e. W is the stationary operand (a fresh [128,128] tile per matmul, loaded
via LDWEIGHTS which pipelines ahead of the running matmul through the PE's
reorder window + FWL) and the moving operand is the unit's token block, so
the matmul cost is proportional to the REAL token count (padded only to the
slot capacity), not to a fixed 128-token tile grid.  The matmuls of an
h-chunk pair alternate between two PSUM banks (back-to-back same-bank
accumulation costs the PE ~28ns/matmul).  W is single-use and streamed
through a 6-slab SBUF pool in exact consumption order; per-core traffic:
W 16.8MB + x ~2.4MB + out ~2.2MB (~60us at 358 GB/s, just under the
~63us of PE work, so the kernel is PE-bound with a thin DMA margin).

Evictions (PSUM f32 -> SBUF f16 copy, no scale needed) alternate between the
scalar and vector engines, one per h-chunk, grouped into output DMAs on the
sync queue, with the last unit tapering to single-h-chunk DMAs so the tail
after the final matmul is one small eviction + one small DMA.

Host combine: partial outputs are [h, token]-major f16; host transposes,
accumulates the 2 K-half partials and the topk expert partials per token in
f32, and emits [4, 256, 2048] f16.
"""

import numpy as np

R, T_TOK, E = 4, 1024, 8
I_PR, H = 1024, 2048
K = R * I_PR            # 4096 contraction
P = 128
N_CORES = 8
NKS = 16                # K-subtiles per K-half unit (2048/128)
NHC = H // P            # 16 h-chunks
CHUNK_CAP = 512         # max tokens per unit (one PSUM bank of fp32)
OG = 4                  # h-chunks per output DMA group
NPAIR = NHC // 2        # h-chunk pairs (PSUM-bank-alternating matmul order)
NWARM = 8               # p-state warmup matmuls (N=512)
NFILL = 14              # fine-grained N=256 bridge matmuls after warmup
W_BUFS = 8              # W pair-slab SBUF pool depth (8 x 1MB)

_prog_cache: dict[tuple, object] = {}


def _new_bacc():
    from concourse import bacc

    return bacc.Bacc(
        "TRN2",
        target_bir_lowering=False,
        debug=False,
        num_devices=N_CORES,
    )


def _build_program(caps: tuple):
    import concourse.mybir as mybir
    import concourse.tile as tile

    f16 = mybir.dt.float16
    f32 = mybir.dt.float32

    nc = _new_bacc()
    U = len(caps)
    xs, ws, os_ = [], [], []
    for s, C in enumerate(caps):
        xs.append(nc.declare_dram_parameter(f"x{s}", [P, NKS * C], f16,
                                            isOutput=False))
        ws.append(nc.declare_dram_parameter(f"w{s}", [NPAIR, P, 2 * H], f16,
                                            isOutput=False))
        os_.append(nc.declare_dram_parameter(f"o{s}", [P, NHC * C], f16,
                                             isOutput=True))

    # output DMA grouping: uniform groups of OG h-chunks, except the last
    # unit which tapers (…,2,1,1) so the final transfer after the last
    # matmul is a single small h-chunk.
    def out_groups(s):
        if s == U - 1 and NHC >= 4:
            full = (NHC - 4) // OG
            gs = [(i * OG, OG) for i in range(full)]
            base = full * OG
            rest = NHC - base          # 4 when NHC % OG == 0
            gs += [(base, 1), (base + 1, 1), (base + 2, rest - 2)]
            return [(o, l) for (o, l) in gs if l > 0]
        return [(i * OG, OG) for i in range(NHC // OG)]

    with tile.TileContext(nc) as tc:
        with tc.tile_pool(name="sb", bufs=1) as sb, \
             tc.tile_pool(name="ps", bufs=2, space="PSUM") as psp:
            # All DMAs ride the sync HWDGE queue (~0.65us serial issue each;
            # SWDGE via scalar/gpsimd costs multi-us descriptor generation),
            # emitted in exact consumption order.  W pair-slabs use the
            # ks-interleaved layout [ks*256 + half*128 + c] so a slab is
            # consumed strictly left-to-right and partial-slab DMAs gate
            # only the matmuls that need them.
            xt = [sb.tile([P, NKS * C], f16, name=f"x{s}", tag=f"x{s}", bufs=1)
                  for s, C in enumerate(caps)]
            wt_of = {}
            ot_of = {}

            def dma_x(s, q, parts=4):
                C = caps[s]
                XQ = NKS * C // parts
                nc.sync.dma_start(xt[s][:, q * XQ:(q + 1) * XQ],
                                  xs[s][:, q * XQ:(q + 1) * XQ])

            def alloc_w(s, pr):
                wt = sb.tile([P, 2 * H], f16, name=f"w{s}_{pr}", tag="w",
                             bufs=W_BUFS)
                wt_of[(s, pr)] = wt
                return wt

            def dma_w_part(wt, s, pr, parts, i):
                WQ = 2 * H // parts
                nc.sync.dma_start(wt[:, i * WQ:(i + 1) * WQ],
                                  ws[s][pr, :, i * WQ:(i + 1) * WQ])

            def dma_w(s, pr):
                wt = alloc_w(s, pr)
                dma_w_part(wt, s, pr, 1, 0)

            def dma_o(s, off, ln):
                C = caps[s]
                nc.sync.dma_start(os_[s][:, off * C:(off + ln) * C],
                                  ot_of[(s, off)][:])

            pairs_global = [(s, pr) for s in range(U) for pr in range(NPAIR)]
            NPG = len(pairs_global)

            # prologue: x unit0 in halves + first five W slabs (0 in
            # quarters, 1 in halves), interleaved so transfers pace with the
            # ramping PE's early consumption.
            w0 = alloc_w(*pairs_global[0])
            dma_x(0, 0)
            dma_w_part(w0, *pairs_global[0], 4, 0)
            dma_w_part(w0, *pairs_global[0], 4, 1)
            dma_x(0, 1)
            dma_w_part(w0, *pairs_global[0], 4, 2)
            dma_x(0, 2)
            dma_w_part(w0, *pairs_global[0], 4, 3)
            if NPG > 1:
                w1 = alloc_w(*pairs_global[1])
                dma_w_part(w1, *pairs_global[1], 2, 0)
                dma_x(0, 3)
                dma_w_part(w1, *pairs_global[1], 2, 1)
            else:
                dma_x(0, 3)
            for p in range(2, min(W_BUFS, NPG)):
                wp_ = alloc_w(*pairs_global[p])
                dma_w_part(wp_, *pairs_global[p], 2, 0)
                dma_w_part(wp_, *pairs_global[p], 2, 1)

            # p-state warmup: occupy the PE (ramping toward full clock)
            # while the first W/x transfers land; zeroed operands.
            warm = sb.tile([P, 512], f16, name="warm", tag="warm", bufs=1)
            nc.vector.memset(warm[:], 0.0)
            ps_warm = psp.tile([P, 512], f32, name="psw", tag="ps", bufs=8)
            for i in range(NWARM):
                nc.tensor.matmul(ps_warm[:, :512], lhsT=warm[:, :P],
                                 rhs=warm[:], start=(i == 0),
                                 stop=(i == NWARM - 1))
            # fine-grained bridge: 0.11us quantization at the point where
            # the first real data lands, so the PE stays busy (p-state) and
            # real matmuls begin with minimal overshoot
            for i in range(NFILL):
                nc.tensor.matmul(ps_warm[:, :256], lhsT=warm[:, :P],
                                 rhs=warm[:, :256], start=True, stop=True)

            # compute: matmuls of a pair alternate between two PSUM banks
            # (back-to-back same-bank accumulation stalls the PE ~28ns/MM).
            # Further W slabs / x tiles / out DMAs are emitted AFTER each
            # pair's matmuls so pool-rotation WAR deps bind to the right
            # readers; the sync queue runs ahead under W-pool backpressure.
            pending_outs = []
            group_of = {s: {} for s in range(U)}
            for s in range(U):
                for (off, ln) in out_groups(s):
                    for i in range(ln):
                        group_of[s][off + i] = (off, ln)

            for t, (s, pr) in enumerate(pairs_global):
                while pending_outs and pending_outs[0][3] < t:
                    so, off, ln, _ = pending_outs.pop(0)
                    dma_o(so, off, ln)
                C = caps[s]
                wt = wt_of[(s, pr)]
                ps_a = psp.tile([P, 512], f32, name=f"psa{s}_{pr}",
                                tag="ps", bufs=8)
                ps_b = psp.tile([P, 512], f32, name=f"psb{s}_{pr}",
                                tag="ps", bufs=8)
                for ks in range(NKS):
                    for half, ps in ((0, ps_a), (1, ps_b)):
                        nc.tensor.matmul(
                            ps[:, :C],
                            lhsT=wt[:, ks * 2 * P + half * P:
                                    ks * 2 * P + (half + 1) * P],
                            rhs=xt[s][:, ks * C:(ks + 1) * C],
                            start=(ks == 0),
                            stop=(ks == NKS - 1),
                        )
                for half, ps in ((0, ps_a), (1, ps_b)):
                    h = 2 * pr + half
                    off, ln = group_of[s][h]
                    if h == off:
                        ot_of[(s, off)] = sb.tile([P, ln * C], f16,
                                                  name=f"o{s}_{off}",
                                                  tag="o", bufs=3)
                    dst = ot_of[(s, off)][:, (h - off) * C:(h - off + 1) * C]
                    if half == 0:
                        nc.scalar.copy(dst, ps[:, :C])
                    else:
                        nc.vector.tensor_scalar_mul(dst, ps[:, :C], 1.0)
                    if h == off + ln - 1:
                        pending_outs.append((s, off, ln, t))
                # stream-ahead issues (emitted after this pair's readers)
                if t + W_BUFS < NPG:
                    wp_ = alloc_w(*pairs_global[t + W_BUFS])
                    dma_w_part(wp_, *pairs_global[t + W_BUFS], 2, 0)
                    dma_w_part(wp_, *pairs_global[t + W_BUFS], 2, 1)
                if s + 1 < U and pr < 4:
                    dma_x(s + 1, pr)
            for so, off, ln, _ in pending_outs:
                dma_o(so, off, ln)
    nc.finalize()
    return nc


def _get_program(caps: tuple):
    if caps not in _prog_cache:
        _prog_cache[caps] = _build_program(caps)
    return _prog_cache[caps]


def _route(logits, topk):
    """numpy replica of jax.lax.top_k + softmax over selected logits."""
    idx = np.argsort(-logits, axis=-1, kind="stable")[:, :topk]      # [T, topk]
    vals = np.take_along_axis(logits, idx, axis=-1)
    mx = vals.max(-1, keepdims=True)
    gate = np.exp(vals - mx)
    gate = gate / gate.sum(-1, keepdims=True)                        # f32
    return idx, gate


def prepare(inputs):
    """Host routing + per-core input construction.

    Returns (nc, launches, combine): launches is a list of per-launch in_maps
    (one dict per core); combine(list_of_per_launch_results) -> final output.
    """
    x = np.asarray(inputs["intermediate_states"])          # [R, TK, I_PR] f16
    w = np.asarray(inputs["w"])                            # [R, E, I_PR, H] f16
    logits = np.asarray(inputs["router_logits"]).astype(np.float32)  # [T, E]
    topk = int(np.asarray(inputs["topk"]))

    T, E_ = logits.shape
    TK = T * topk
    assert x.shape == (R, TK, I_PR) and w.shape == (R, E_, I_PR, H) and E_ == E

    idx, gate = _route(logits, topk)
    flat_e = idx.reshape(-1)                               # expert of tk
    g_flat = gate.reshape(TK)
    counts = np.bincount(flat_e, minlength=E)
    starts = np.zeros(E + 1, np.int64)
    starts[1:] = np.cumsum(counts)
    order = np.argsort(flat_e, kind="stable")              # tks sorted by expert

    # pre-gated activations: y contribution of token row is gate * x row
    xf = np.ascontiguousarray(x.transpose(1, 0, 2)).reshape(TK, K)
    xg = (xf.astype(np.float32) * g_flat[:, None]).astype(np.float16)

    # build units: (expert, kh, token array)
    units = []
    for e in range(E):
        toks_e = order[starts[e]:starts[e + 1]]
        for lo in range(0, max(len(toks_e), 1), CHUNK_CAP):
            chunk = toks_e[lo:lo + CHUNK_CAP]
            for kh in range(2):
                units.append((e, kh, chunk))
    while len(units) % N_CORES:
        units.append((0, 0, np.empty(0, np.int64)))
    units.sort(key=lambda u: -len(u[2]))
    U = len(units) // N_CORES

    caps = []
    for s in range(U):
        cmax = max(len(u[2]) for u in units[s * N_CORES:(s + 1) * N_CORES])
        caps.append(max(8, -(-cmax // 8) * 8))
    caps = tuple(caps)
    nc = _get_program(caps)

    KH = K // 2
    in_maps = [dict() for _ in range(N_CORES)]
    unit_of = {}                                           # (core, slot) -> unit
    for s, C in enumerate(caps):
        for c_core in range(N_CORES):
            e, kh, toks = units[s * N_CORES + c_core]
            n = len(toks)
            unit_of[(c_core, s)] = (toks, C)
            x_pack = np.zeros((P, NKS, C), np.float16)
            if n:
                sub = xg[toks, kh * KH:(kh + 1) * KH]      # [n, 2048]
                x_pack[:, :, :n] = sub.reshape(n, NKS, P).transpose(2, 1, 0)
            if n:
                W_kh = w[2 * kh:2 * kh + 2, e].reshape(KH, H)
                w_pack = np.ascontiguousarray(
                    W_kh.reshape(NKS, P, NPAIR, 2, P)
                    .transpose(2, 1, 0, 3, 4)       # [pair, kpart, ks, half, hcol]
                ).reshape(NPAIR, P, 2 * H)
            else:
                w_pack = np.zeros((NPAIR, P, 2 * H), np.float16)
            in_maps[c_core][f"x{s}"] = x_pack.reshape(P, NKS * C)
            in_maps[c_core][f"w{s}"] = w_pack

    launches = [in_maps]

    def combine(all_results):
        res = all_results[0]
        y2 = np.zeros((TK, H), np.float32)
        for (c_core, s), (toks, C) in unit_of.items():
            n = len(toks)
            if not n:
                continue
            o_u = res[c_core][f"o{s}"]                     # [P, NHC*C] f16
            part = o_u.reshape(P, NHC, C)[:, :, :n]
            y2[toks] += part.transpose(2, 1, 0).reshape(n, H)
        y = y2.reshape(T, topk, H).sum(axis=1)
        return y.astype(np.float16).reshape(R, T // R, H)

    return nc, launches, combine


def kernel(**inputs) -> np.ndarray:
    nc, launches, combine = prepare(inputs)
    from concourse.bass_utils import run_bass_kernel_spmd

    all_results = []
    for in_maps in launches:
        res = run_bass_kernel_spmd(nc, in_maps, core_ids=list(range(N_CORES)))
        all_results.append(res.results)
    return combine(all_results)


# revision 20
# speedup vs baseline: 1.1488x; 1.1488x over previous
"""MoE top-k routing + grouped down-proj GEMM + reduce-scatter for trn2 (8 cores).

Problem: intermediate_states [4, 2048, 1024] f16 (rank-sharded expanded-token
activations), w [4, 8, 1024, 2048] f16 (rank-sharded per-expert down-proj),
router_logits [1024, 8] f32, topk=2.  Output [4, 256, 2048] f16.

Strategy (W-stationary streaming): per expanded token tk routed to expert
e(tk): y[tk] = gate(tk) * (x_full[tk] @ W_full[e(tk)]) with x_full [TK, 4096]
(rank dim folded into contraction) and W_full[e] [4096, 2048].  Gates are
pre-applied to x on the host (f32 multiply, cast to f16), so the device does
pure GEMM.

Work unit = (expert, K-half, token-chunk<=512).  For 8 experts with c_e<=512
that is 16 units.  Units are sorted by token count and packed into U=2 "slots"
of 8 (one unit per core per slot); each slot has a common padded capacity C_s
= max count in slot, so all 8 cores run an IDENTICAL program (SPMD) perfectly
balanced by construction.  Per unit the PE runs, for each of 16 h-chunks of
128 output columns, a 16-step accumulation over K-subtiles:

    matmul(psum[128h, C], lhsT=W[128k, 128h], rhs=x[128k, C])

i.e. W is the stationary operand (a fresh [128,128] tile per matmul, loaded
via LDWEIGHTS which pipelines ahead of the running matmul through the PE's
reorder window + FWL) and the moving operand is the unit's token block, so
the matmul cost is proportional to the REAL token count (padded only to the
slot capacity), not to a fixed 128-token tile grid.  The matmuls of an
h-chunk pair alternate between two PSUM banks (back-to-back same-bank
accumulation costs the PE ~28ns/matmul).  W is single-use and streamed
through a 6-slab SBUF pool in exact consumption order; per-core traffic:
W 16.8MB + x ~2.4MB + out ~2.2MB (~60us at 358 GB/s, just under the
~63us of PE work, so the kernel is PE-bound with a thin DMA margin).

Evictions (PSUM f32 -> SBUF f16 copy, no scale needed) alternate between the
scalar and vector engines, one per h-chunk, grouped into output DMAs on the
sync queue, with the last unit tapering to single-h-chunk DMAs so the tail
after the final matmul is one small eviction + one small DMA.

Host combine: partial outputs are [h, token]-major f16; host transposes,
accumulates the 2 K-half partials and the topk expert partials per token in
f32, and emits [4, 256, 2048] f16.
"""

import numpy as np

R, T_TOK, E = 4, 1024, 8
I_PR, H = 1024, 2048
K = R * I_PR            # 4096 contraction
P = 128
N_CORES = 8
NKS = 16                # K-subtiles per K-half unit (2048/128)
NHC = H // P            # 16 h-chunks
CHUNK_CAP = 512         # max tokens per unit (one PSUM bank of fp32)
OG = 4                  # h-chunks per output DMA group
NPAIR = NHC // 2        # h-chunk pairs (PSUM-bank-alternating matmul order)
NWARM = 8               # p-state warmup matmuls (N=512)
NFILL = 14              # fine-grained N=256 bridge matmuls after warmup
W_BUFS = 6              # W pair-slab SBUF pool depth (6 x 1MB)

_prog_cache: dict[tuple, object] = {}


def _new_bacc():
    from concourse import bacc

    return bacc.Bacc(
        "TRN2",
        target_bir_lowering=False,
        debug=False,
        num_devices=N_CORES,
    )


def _build_program(caps: tuple):
    import concourse.mybir as mybir
    import concourse.tile as tile

    f16 = mybir.dt.float16
    f32 = mybir.dt.float32

    nc = _new_bacc()
    U = len(caps)
    xs, ws, os_ = [], [], []
    for s, C in enumerate(caps):
        xs.append(nc.declare_dram_parameter(f"x{s}", [P, NKS * C], f16,
                                            isOutput=False))
        ws.append(nc.declare_dram_parameter(f"w{s}", [NPAIR, P, 2 * H], f16,
                                            isOutput=False))
        os_.append(nc.declare_dram_parameter(f"o{s}", [P, NHC * C], f16,
                                             isOutput=True))

    # output DMA grouping: uniform groups of OG h-chunks, except the last
    # unit which tapers (…,2,1,1) so the final transfer after the last
    # matmul is a single small h-chunk.
    def out_groups(s):
        if s == U - 1 and NHC >= 4:
            full = (NHC - 4) // OG
            gs = [(i * OG, OG) for i in range(full)]
            base = full * OG
            rest = NHC - base          # 4 when NHC % OG == 0
            gs += [(base, 1), (base + 1, 1), (base + 2, rest - 2)]
            return [(o, l) for (o, l) in gs if l > 0]
        return [(i * OG, OG) for i in range(NHC // OG)]

    with tile.TileContext(nc) as tc:
        with tc.tile_pool(name="sb", bufs=1) as sb, \
             tc.tile_pool(name="ps", bufs=2, space="PSUM") as psp:
            # All DMAs ride the sync HWDGE queue (~0.65us serial issue each;
            # SWDGE via scalar/gpsimd costs multi-us descriptor generation),
            # emitted in exact consumption order.  W pair-slabs use the
            # ks-interleaved layout [ks*256 + half*128 + c] so a slab is
            # consumed strictly left-to-right and partial-slab DMAs gate
            # only the matmuls that need them.
            xt = [sb.tile([P, NKS * C], f16, name=f"x{s}", tag=f"x{s}", bufs=1)
                  for s, C in enumerate(caps)]
            wt_of = {}
            ot_of = {}

            def dma_x(s, q, parts=4):
                C = caps[s]
                XQ = NKS * C // parts
                nc.sync.dma_start(xt[s][:, q * XQ:(q + 1) * XQ],
                                  xs[s][:, q * XQ:(q + 1) * XQ])

            def alloc_w(s, pr):
                wt = sb.tile([P, 2 * H], f16, name=f"w{s}_{pr}", tag="w",
                             bufs=W_BUFS)
                wt_of[(s, pr)] = wt
                return wt

            def dma_w_part(wt, s, pr, parts, i):
                WQ = 2 * H // parts
                nc.sync.dma_start(wt[:, i * WQ:(i + 1) * WQ],
                                  ws[s][pr, :, i * WQ:(i + 1) * WQ])

            def dma_w(s, pr):
                wt = alloc_w(s, pr)
                dma_w_part(wt, s, pr, 1, 0)

            def dma_o(s, off, ln):
                C = caps[s]
                nc.sync.dma_start(os_[s][:, off * C:(off + ln) * C],
                                  ot_of[(s, off)][:])

            pairs_global = [(s, pr) for s in range(U) for pr in range(NPAIR)]
            NPG = len(pairs_global)

            # prologue: x unit0 in halves + first five W slabs (0 in
            # quarters, 1 in halves), interleaved so transfers pace with the
            # ramping PE's early consumption.
            w0 = alloc_w(*pairs_global[0])
            dma_x(0, 0)
            dma_w_part(w0, *pairs_global[0], 4, 0)
            dma_w_part(w0, *pairs_global[0], 4, 1)
            dma_x(0, 1)
            dma_w_part(w0, *pairs_global[0], 4, 2)
            dma_x(0, 2)
            dma_w_part(w0, *pairs_global[0], 4, 3)
            if NPG > 1:
                w1 = alloc_w(*pairs_global[1])
                dma_w_part(w1, *pairs_global[1], 2, 0)
                dma_x(0, 3)
                dma_w_part(w1, *pairs_global[1], 2, 1)
            else:
                dma_x(0, 3)
            for p in range(2, min(W_BUFS, NPG)):
                wp_ = alloc_w(*pairs_global[p])
                dma_w_part(wp_, *pairs_global[p], 2, 0)
                dma_w_part(wp_, *pairs_global[p], 2, 1)

            # p-state warmup: occupy the PE (ramping toward full clock)
            # while the first W/x transfers land; zeroed operands.
            warm = sb.tile([P, 512], f16, name="warm", tag="warm", bufs=1)
            nc.vector.memset(warm[:], 0.0)
            ps_warm = psp.tile([P, 512], f32, name="psw", tag="ps", bufs=8)
            for i in range(NWARM):
                nc.tensor.matmul(ps_warm[:, :512], lhsT=warm[:, :P],
                                 rhs=warm[:], start=(i == 0),
                                 stop=(i == NWARM - 1))
            # fine-grained bridge: 0.11us quantization at the point where
            # the first real data lands, so the PE stays busy (p-state) and
            # real matmuls begin with minimal overshoot
            for i in range(NFILL):
                nc.tensor.matmul(ps_warm[:, :256], lhsT=warm[:, :P],
                                 rhs=warm[:, :256], start=True, stop=True)

            # compute: matmuls of a pair alternate between two PSUM banks
            # (back-to-back same-bank accumulation stalls the PE ~28ns/MM).
            # Further W slabs / x tiles / out DMAs are emitted AFTER each
            # pair's matmuls so pool-rotation WAR deps bind to the right
            # readers; the sync queue runs ahead under W-pool backpressure.
            pending_outs = []
            group_of = {s: {} for s in range(U)}
            for s in range(U):
                for (off, ln) in out_groups(s):
                    for i in range(ln):
                        group_of[s][off + i] = (off, ln)

            for t, (s, pr) in enumerate(pairs_global):
                while pending_outs and pending_outs[0][3] < t:
                    so, off, ln, _ = pending_outs.pop(0)
                    dma_o(so, off, ln)
                C = caps[s]
                wt = wt_of[(s, pr)]
                ps_a = psp.tile([P, 512], f32, name=f"psa{s}_{pr}",
                                tag="ps", bufs=8)
                ps_b = psp.tile([P, 512], f32, name=f"psb{s}_{pr}",
                                tag="ps", bufs=8)
                for ks in range(NKS):
                    for half, ps in ((0, ps_a), (1, ps_b)):
                        nc.tensor.matmul(
                            ps[:, :C],
                            lhsT=wt[:, ks * 2 * P + half * P:
                                    ks * 2 * P + (half + 1) * P],
                            rhs=xt[s][:, ks * C:(ks + 1) * C],
                            start=(ks == 0),
                            stop=(ks == NKS - 1),
                        )
                for half, ps in ((0, ps_a), (1, ps_b)):
                    h = 2 * pr + half
                    off, ln = group_of[s][h]
                    if h == off:
                        ot_of[(s, off)] = sb.tile([P, ln * C], f16,
                                                  name=f"o{s}_{off}",
                                                  tag="o", bufs=3)
                    dst = ot_of[(s, off)][:, (h - off) * C:(h - off + 1) * C]
                    if half == 0:
                        nc.scalar.copy(dst, ps[:, :C])
                    else:
                        nc.vector.tensor_scalar_mul(dst, ps[:, :C], 1.0)
                    if h == off + ln - 1:
                        pending_outs.append((s, off, ln, t))
                # stream-ahead issues (emitted after this pair's readers)
                if t + W_BUFS < NPG:
                    wp_ = alloc_w(*pairs_global[t + W_BUFS])
                    dma_w_part(wp_, *pairs_global[t + W_BUFS], 2, 0)
                    dma_w_part(wp_, *pairs_global[t + W_BUFS], 2, 1)
                if s + 1 < U and pr < 4:
                    dma_x(s + 1, pr)
            for so, off, ln, _ in pending_outs:
                dma_o(so, off, ln)
    nc.finalize()
    return nc


def _get_program(caps: tuple):
    if caps not in _prog_cache:
        _prog_cache[caps] = _build_program(caps)
    return _prog_cache[caps]


def _route(logits, topk):
    """numpy replica of jax.lax.top_k + softmax over selected logits."""
    idx = np.argsort(-logits, axis=-1, kind="stable")[:, :topk]      # [T, topk]
    vals = np.take_along_axis(logits, idx, axis=-1)
    mx = vals.max(-1, keepdims=True)
    gate = np.exp(vals - mx)
    gate = gate / gate.sum(-1, keepdims=True)                        # f32
    return idx, gate


def prepare(inputs):
    """Host routing + per-core input construction.

    Returns (nc, launches, combine): launches is a list of per-launch in_maps
    (one dict per core); combine(list_of_per_launch_results) -> final output.
    """
    x = np.asarray(inputs["intermediate_states"])          # [R, TK, I_PR] f16
    w = np.asarray(inputs["w"])                            # [R, E, I_PR, H] f16
    logits = np.asarray(inputs["router_logits"]).astype(np.float32)  # [T, E]
    topk = int(np.asarray(inputs["topk"]))

    T, E_ = logits.shape
    TK = T * topk
    assert x.shape == (R, TK, I_PR) and w.shape == (R, E_, I_PR, H) and E_ == E

    idx, gate = _route(logits, topk)
    flat_e = idx.reshape(-1)                               # expert of tk
    g_flat = gate.reshape(TK)
    counts = np.bincount(flat_e, minlength=E)
    starts = np.zeros(E + 1, np.int64)
    starts[1:] = np.cumsum(counts)
    order = np.argsort(flat_e, kind="stable")              # tks sorted by expert

    # pre-gated activations: y contribution of token row is gate * x row
    xf = np.ascontiguousarray(x.transpose(1, 0, 2)).reshape(TK, K)
    xg = (xf.astype(np.float32) * g_flat[:, None]).astype(np.float16)

    # build units: (expert, kh, token array)
    units = []
    for e in range(E):
        toks_e = order[starts[e]:starts[e + 1]]
        for lo in range(0, max(len(toks_e), 1), CHUNK_CAP):
            chunk = toks_e[lo:lo + CHUNK_CAP]
            for kh in range(2):
                units.append((e, kh, chunk))
    while len(units) % N_CORES:
        units.append((0, 0, np.empty(0, np.int64)))
    units.sort(key=lambda u: -len(u[2]))
    U = len(units) // N_CORES

    caps = []
    for s in range(U):
        cmax = max(len(u[2]) for u in units[s * N_CORES:(s + 1) * N_CORES])
        caps.append(max(8, -(-cmax // 8) * 8))
    caps = tuple(caps)
    nc = _get_program(caps)

    KH = K // 2
    in_maps = [dict() for _ in range(N_CORES)]
    unit_of = {}                                           # (core, slot) -> unit
    for s, C in enumerate(caps):
        for c_core in range(N_CORES):
            e, kh, toks = units[s * N_CORES + c_core]
            n = len(toks)
            unit_of[(c_core, s)] = (toks, C)
            x_pack = np.zeros((P, NKS, C), np.float16)
            if n:
                sub = xg[toks, kh * KH:(kh + 1) * KH]      # [n, 2048]
                x_pack[:, :, :n] = sub.reshape(n, NKS, P).transpose(2, 1, 0)
            if n:
                W_kh = w[2 * kh:2 * kh + 2, e].reshape(KH, H)
                w_pack = np.ascontiguousarray(
                    W_kh.reshape(NKS, P, NPAIR, 2, P)
                    .transpose(2, 1, 0, 3, 4)       # [pair, kpart, ks, half, hcol]
                ).reshape(NPAIR, P, 2 * H)
            else:
                w_pack = np.zeros((NPAIR, P, 2 * H), np.float16)
            in_maps[c_core][f"x{s}"] = x_pack.reshape(P, NKS * C)
            in_maps[c_core][f"w{s}"] = w_pack

    launches = [in_maps]

    def combine(all_results):
        res = all_results[0]
        y2 = np.zeros((TK, H), np.float32)
        for (c_core, s), (toks, C) in unit_of.items():
            n = len(toks)
            if not n:
                continue
            o_u = res[c_core][f"o{s}"]                     # [P, NHC*C] f16
            part = o_u.reshape(P, NHC, C)[:, :, :n]
            y2[toks] += part.transpose(2, 1, 0).reshape(n, H)
        y = y2.reshape(T, topk, H).sum(axis=1)
        return y.astype(np.float16).reshape(R, T // R, H)

    return nc, launches, combine


def kernel(**inputs) -> np.ndarray:
    nc, launches, combine = prepare(inputs)
    from concourse.bass_utils import run_bass_kernel_spmd

    all_results = []
    for in_maps in launches:
        res = run_bass_kernel_spmd(nc, in_maps, core_ids=list(range(N_CORES)))
        all_results.append(res.results)
    return combine(all_results)
